# revision 8
# baseline (speedup 1.0000x reference)
"""AxialAttention TRN2 Bass kernel — 8-core data-parallel over batch (v2, bf16).

Reference math (per batch element b, per head h):
  qkv = x @ w_qkv;  q,k,v split; heads of dh=64
  S[m, n] = q_m . k_n / 8   (m, n over 1024 = 32x32 positions)
  attn = softmax over y only, where n = x*32 + y  (groups of 32 consecutive n)
  out[m] = sum_n attn[m, n] v[n];  y = out @ w_out + b_out

v2 changes vs v1:
  - bf16 operands everywhere on the matmul path (host converts inputs);
    PSUM accumulation stays f32.
  - Z group-sums matmul directly into sel-strip layout (i4big weights):
    no rz relocation DMAs, no rz dtype-copy (bitcast f32->f32r instead).
  - pv accumulates all 4 (hh, mc) quadrants of a head-pair in one
    [128,1024] PSUM tile; single ACT copyout -> outT (no stage merges).
  - bias b_out applied on host; y emitted transposed [DIM, M] and
    transposed back on host.
  - front PSUM->SBUF copies on ACT (DVE reserved for attention mults).

Per-core layout:
  xT   4x[128 k, 1024 m] bf16 (PE transpose of x)
  qkT  8x[128 f, 1024 m] bf16 = (x @ w_qkv[:, :1024]).T
  v    8x[128 pos, 512 vf] bf16
  E^T  per (s, hh, nt): [128 n, 1024 m] bf16 = exp(S^T/8)
  Z    strips [128, 512] f32 per (hh, mc, r): group sums in sel-strip rows
  R    [128 n, 512 m] f32 PSUM via sel matmuls; E' = E * R (DVE)
  outT 4x[128 (2 heads x dh), 1024 m] bf16
  yT   [512 dim, 1024 m] f32 -> DRAM; host transposes + adds bias
"""
import numpy as np

B, H, W, DIM = 8, 32, 32, 512
HEADS, DH = 8, 64
M = H * W          # 1024 query/key positions
NT = M // 128      # 8 n-tiles / m-tiles / pos-tiles

_CACHE = {}


def _build(loop_n=1):
    import concourse.bass as bass
    import concourse.mybir as mybir
    import concourse.tile as tile
    from concourse import bacc
    from contextlib import ExitStack

    F32 = mybir.dt.float32
    F32R = mybir.dt.float32r
    BF16 = mybir.dt.bfloat16
    EXP = mybir.ActivationFunctionType.Exp
    COPY = mybir.ActivationFunctionType.Copy

    nc = bacc.Bacc("TRN2", target_bir_lowering=False, debug=False,
                   enable_asserts=False, num_devices=8)
    x = nc.dram_tensor("x", [M, DIM], BF16, kind="ExternalInput").ap()
    w_qkv = nc.dram_tensor("w_qkv", [DIM, 3 * DIM], BF16, kind="ExternalInput").ap()
    w_out = nc.dram_tensor("w_out", [DIM, DIM], BF16, kind="ExternalInput").ap()
    ident = nc.dram_tensor("ident", [128, 128], BF16, kind="ExternalInput").ap()
    i4big = nc.dram_tensor("i4big", [128, 512], BF16, kind="ExternalInput").ap()
    sel = nc.dram_tensor("sel", [128, 128], BF16, kind="ExternalInput").ap()
    y = nc.dram_tensor("y", [DIM, M], F32, kind="ExternalOutput").ap()

    with tile.TileContext(nc) as tc, ExitStack() as top:
        if loop_n > 1:
            top.enter_context(tc.For_i(0, loop_n, 1))
        persist = top.enter_context(tc.tile_pool(name="persist", bufs=1))

        # ---- persistent constants ----
        ident_sb = persist.tile([128, 128], BF16, tag="ident")
        nc.sync.dma_start(out=ident_sb, in_=ident)
        i4_sb = persist.tile([128, 512], BF16, tag="i4")
        nc.sync.dma_start(out=i4_sb, in_=i4big)
        sel_sb = persist.tile([128, 128], BF16, tag="sel")
        nc.sync.dma_start(out=sel_sb, in_=sel)
        wo_sb = [persist.tile([128, DIM], BF16, tag=f"wo{i}", name=f"wo{i}") for i in range(4)]
        for i in range(4):
            nc.sync.dma_start(out=wo_sb[i], in_=w_out[128 * i:128 * (i + 1), :])

        # ---- persistent activations ----
        qkT_sb = [persist.tile([128, M], BF16, tag=f"qkT{i}", name=f"qkT{i}") for i in range(8)]
        v_sb = [persist.tile([128, DIM], BF16, tag=f"v{i}", name=f"v{i}") for i in range(NT)]
        outT_sb = [persist.tile([128, M], BF16, tag=f"outT{i}", name=f"outT{i}") for i in range(4)]

        # ================= FRONT =================
        with ExitStack() as fr:
            fsb = fr.enter_context(tc.tile_pool(name="front_sb", bufs=1))
            xt_pool = fr.enter_context(tc.tile_pool(name="xt_ps", bufs=2, space="PSUM"))
            mm_pool = fr.enter_context(tc.tile_pool(name="mm_ps", bufs=2, space="PSUM"))
            vp_pool = fr.enter_context(tc.tile_pool(name="vp_ps", bufs=2, space="PSUM"))

            x_sb = [fsb.tile([128, DIM], BF16, tag=f"x{mt}", name=f"x{mt}") for mt in range(NT)]
            for mt in range(NT):
                nc.sync.dma_start(out=x_sb[mt], in_=x[128 * mt:128 * (mt + 1), :])
            wq_sb = [fsb.tile([128, 3 * DIM], BF16, tag=f"wq{kt}", name=f"wq{kt}") for kt in range(4)]
            for kt in range(4):
                nc.sync.dma_start(out=wq_sb[kt], in_=w_qkv[128 * kt:128 * (kt + 1), :])

            # x transpose: xT[kc] [128 k, 1024 m]
            xT_sb = []
            for kc in range(4):
                xt_ps = xt_pool.tile([128, M], BF16, tag="xt")
                for mt in range(NT):
                    nc.tensor.matmul(xt_ps[:, 128 * mt:128 * (mt + 1)],
                                     x_sb[mt][:, 128 * kc:128 * (kc + 1)],
                                     ident_sb, is_transpose=True,
                                     start=True, stop=True)
                t = fsb.tile([128, M], BF16, tag=f"xT{kc}", name=f"xT{kc}")
                nc.scalar.activation(out=t, in_=xt_ps, func=COPY)
                xT_sb.append(t)

            # qkT[ft] = (x @ w_qkv[:, :1024]).T f-tile ft
            # heads (2s, 2s+1) need ft s (q) and 4+s (k): emit s-pair order
            for ft in (0, 4, 1, 5, 2, 6, 3, 7):
                qk_ps = mm_pool.tile([128, M], F32, tag="mm")
                for mc in range(2):
                    for kt in range(4):
                        nc.tensor.matmul(
                            qk_ps[:, 512 * mc:512 * (mc + 1)],
                            wq_sb[kt][:, 128 * ft:128 * (ft + 1)],
                            xT_sb[kt][:, 512 * mc:512 * (mc + 1)],
                            start=(kt == 0), stop=(kt == 3))
                nc.scalar.activation(out=qkT_sb[ft], in_=qk_ps, func=COPY)

            # v natural: v[pt] [128 pos, 512 vf]
            for pt in range(NT):
                v_ps = vp_pool.tile([128, DIM], F32, tag="vp")
                for kt in range(4):
                    nc.tensor.matmul(v_ps,
                                     xT_sb[kt][:, 128 * pt:128 * (pt + 1)],
                                     wq_sb[kt][:, 1024:1536],
                                     start=(kt == 0), stop=(kt == 3))
                nc.scalar.activation(out=v_sb[pt], in_=v_ps, func=COPY)

        # ================= ATTENTION =================
        with ExitStack() as at:
            e_pool = at.enter_context(tc.tile_pool(name="e_sb", bufs=24))
            strip_pool = at.enter_context(tc.tile_pool(name="strip_sb", bufs=4))
            s_pool = at.enter_context(tc.tile_pool(name="s_ps", bufs=2, space="PSUM"))
            zr_pool = at.enter_context(tc.tile_pool(name="zr_ps", bufs=2, space="PSUM"))
            pv_pool = at.enter_context(tc.tile_pool(name="pv_ps", bufs=1, space="PSUM"))

            E = {}       # (s, hh, nt) -> tile
            pv_tiles = {}

            def alpha_chunks(s):
                out = []
                for nt in range(NT):
                    for hh in range(2):
                        def emit(s=s, nt=nt, hh=hh):
                            h = 2 * s + hh
                            off = 64 * hh
                            qt = qkT_sb[h // 2]
                            kt_ = qkT_sb[4 + h // 2]
                            s_ps = s_pool.tile([128, M], F32, tag="s",
                                               name=f"sps_{s}_{nt}_{hh}")
                            for mc in range(2):
                                nc.tensor.matmul(
                                    s_ps[:, 512 * mc:512 * (mc + 1)],
                                    kt_[off:off + 64, 128 * nt:128 * (nt + 1)],
                                    qt[off:off + 64, 512 * mc:512 * (mc + 1)],
                                    start=True, stop=True,
                                    tile_position=(off, 0))
                            e = e_pool.tile([128, M], BF16, tag="E",
                                            name=f"E_{s}_{nt}_{hh}")
                            nc.scalar.activation(out=e, in_=s_ps, func=EXP,
                                                 scale=0.125)
                            E[s, hh, nt] = e
                        out.append(emit)
                return out

            def beta_chunks(s):
                chunks = []

                def mkpv(s=s):
                    pv_tiles[s] = pv_pool.tile([128, M], F32, tag="pv",
                                               name=f"pv_{s}")
                chunks.append(mkpv)

                for hh in range(2):
                    for mc in range(2):
                        unit = {}

                        def zblock(s=s, hh=hh, mc=mc, unit=unit):
                            mcs = slice(512 * mc, 512 * (mc + 1))
                            zps = [zr_pool.tile([32 * 4, 512], F32, tag="zr",
                                                name=f"z_{s}_{hh}_{mc}_{r}")
                                   for r in range(2)]
                            for nt in range(NT):
                                a = nt % 4
                                nc.tensor.matmul(
                                    zps[nt // 4],
                                    i4_sb[:, 128 * a:128 * (a + 1)],
                                    E[s, hh, nt][:, mcs],
                                    start=(a == 0), stop=(a == 3))
                            strips = []
                            for r in range(2):
                                st = strip_pool.tile([128, 512], BF16, tag="strip",
                                                     name=f"strip_{s}_{hh}_{mc}_{r}")
                                # reciprocal_approx_fast with a bf16 output
                                # (wrapper asserts f32-out; the op itself
                                # casts on the final write)
                                from concourse.dve_ops import (
                                    RECIP_APPROX_FAST_CONSTS,
                                    RECIPROCAL_APPROX_FAST,
                                )
                                c = RECIP_APPROX_FAST_CONSTS
                                nc.vector._custom_dve(
                                    RECIPROCAL_APPROX_FAST,
                                    out=st, in0=zps[r],
                                    s0=c["s0"], s1=c["s1"], imm2=c["imm2"])
                                strips.append(st)
                            unit["strips"] = strips
                        chunks.append(zblock)

                        for nt in range(NT):
                            def step(s=s, hh=hh, mc=mc, nt=nt, unit=unit):
                                h = 2 * s + hh
                                mcs = slice(512 * mc, 512 * (mc + 1))
                                r, a = nt // 4, nt % 4
                                r_ps = zr_pool.tile([128, 512], F32, tag="zr",
                                                    name=f"rps_{s}_{hh}_{mc}_{nt}")
                                strip = unit["strips"][r]
                                nc.tensor.matmul(
                                    r_ps, sel_sb[32 * a:32 * a + 4, :],
                                    strip[32 * a:32 * a + 4, :],
                                    start=True, stop=True,
                                    tile_position=(32 * a, 0))
                                nc.vector.tensor_mul(
                                    out=E[s, hh, nt][:, mcs],
                                    in0=E[s, hh, nt][:, mcs], in1=r_ps)
                                nc.tensor.matmul(
                                    pv_tiles[s][64 * hh:64 * (hh + 1), mcs],
                                    v_sb[nt][:, 64 * h:64 * (h + 1)],
                                    E[s, hh, nt][:, mcs],
                                    start=(nt == 0), stop=(nt == 7))
                            chunks.append(step)

                def copyout(s=s):
                    nc.scalar.activation(out=outT_sb[s], in_=pv_tiles[s],
                                         func=COPY)
                chunks.append(copyout)
                return chunks

            # software-pipeline: alpha(s) interleaved with beta(s-1)
            npair = 4
            for s in range(npair + 1):
                a = alpha_chunks(s) if s < npair else []
                b = beta_chunks(s - 1) if s >= 1 else []
                na, nb = len(a), len(b)
                if not a:
                    for f in b:
                        f()
                else:
                    ratio = nb / na if na else 0
                    bi = 0.0
                    for i, f in enumerate(a):
                        f()
                        target = (i + 1) * ratio
                        while bi < target and int(bi) < nb:
                            b[int(bi)]()
                            bi += 1
                    for j in range(int(bi), nb):
                        b[j]()

        # ================= PROJ (yT = w_out.T-contract @ outT) =================
        with ExitStack() as pj:
            pj_pool = pj.enter_context(tc.tile_pool(name="pj_ps", bufs=2, space="PSUM"))
            y_pool = pj.enter_context(tc.tile_pool(name="y_sb", bufs=2))
            for dc in range(4):
                y_sb = y_pool.tile([128, M], F32, tag="y")
                for mc in range(2):
                    p = pj_pool.tile([128, 512], F32, tag="pj")
                    for kt in range(4):
                        nc.tensor.matmul(p,
                                         wo_sb[kt][:, 128 * dc:128 * (dc + 1)],
                                         outT_sb[kt][:, 512 * mc:512 * (mc + 1)],
                                         start=(kt == 0), stop=(kt == 3))
                    nc.scalar.activation(out=y_sb[:, 512 * mc:512 * (mc + 1)],
                                         in_=p, func=COPY)
                nc.sync.dma_start(out=y[128 * dc:128 * (dc + 1), :], in_=y_sb)

    nc.compile()
    return nc


def _consts():
    import ml_dtypes
    bf16 = ml_dtypes.bfloat16
    ident = np.eye(128, dtype=np.float32).astype(bf16)
    # i4big[:, 128a:128(a+1)] maps E-tile partition p -> strip row 32a + p//32
    i4big = np.zeros((128, 512), np.float32)
    for a in range(4):
        for p in range(128):
            i4big[p, 128 * a + 32 * a + p // 32] = 1.0
    i4big_bf = i4big.astype(bf16)
    sel = np.zeros((128, 128), np.float32)
    for p in range(128):
        if p % 32 < 4:
            for c in range(128):
                if c // 32 == p % 32:
                    sel[p, c] = 1.0
    return ident, i4big_bf, sel.astype(bf16)


def _in_maps(x, w_qkv, w_out):
    import ml_dtypes
    bf16 = ml_dtypes.bfloat16
    ident, i4big, sel = _consts()
    x = np.asarray(x, dtype=np.float32)
    wq = np.asarray(w_qkv, np.float32).astype(bf16)
    wo = np.asarray(w_out, np.float32).astype(bf16)
    maps = []
    for c in range(8):
        maps.append({
            "x": np.ascontiguousarray(x[c].reshape(M, DIM)).astype(bf16),
            "w_qkv": wq, "w_out": wo,
            "ident": ident, "i4big": i4big, "sel": sel,
        })
    return maps


def kernel(x, w_qkv, w_out, b_out):
    from concourse import bass_utils
    if "nc" not in _CACHE:
        _CACHE["nc"] = _build()
    nc = _CACHE["nc"]
    in_maps = _in_maps(x, w_qkv, w_out)
    res = bass_utils.run_bass_kernel_spmd(nc, in_maps, core_ids=list(range(8)))
    b = np.asarray(b_out, np.float32).reshape(1, DIM)
    out = np.stack([
        (res.results[c]["y"].T + b).reshape(H, W, DIM) for c in range(8)
    ])
    return out


# revision 44
# speedup vs baseline: 1.9998x; 1.9998x over previous
"""AxialAttention TRN2 Bass kernel — 8-core data-parallel over batch (v2, bf16).

Reference math (per batch element b, per head h):
  qkv = x @ w_qkv;  q,k,v split; heads of dh=64
  S[m, n] = q_m . k_n / 8   (m, n over 1024 = 32x32 positions)
  attn = softmax over y only, where n = x*32 + y  (groups of 32 consecutive n)
  out[m] = sum_n attn[m, n] v[n];  y = out @ w_out + b_out

v2 changes vs v1:
  - bf16 operands everywhere on the matmul path (host converts inputs);
    PSUM accumulation stays f32.
  - Z group-sums matmul directly into sel-strip layout (i4big weights):
    no rz relocation DMAs, no rz dtype-copy (bitcast f32->f32r instead).
  - pv accumulates all 4 (hh, mc) quadrants of a head-pair in one
    [128,1024] PSUM tile; single ACT copyout -> outT (no stage merges).
  - bias b_out applied on host; y emitted transposed [DIM, M] and
    transposed back on host.
  - front PSUM->SBUF copies on ACT (DVE reserved for attention mults).

Per-core layout:
  xT   4x[128 k, 1024 m] bf16 (PE transpose of x)
  qkT  8x[128 f, 1024 m] bf16 = (x @ w_qkv[:, :1024]).T
  v    8x[128 pos, 512 vf] bf16
  E^T  per (s, hh, nt): [128 n, 1024 m] bf16 = exp(S^T/8)
  Z    strips [128, 512] f32 per (hh, mc, r): group sums in sel-strip rows
  R    [128 n, 512 m] f32 PSUM via sel matmuls; E' = E * R (DVE)
  outT 4x[128 (2 heads x dh), 1024 m] bf16
  yT   [512 dim, 1024 m] f32 -> DRAM; host transposes + adds bias
"""
import numpy as np

B, H, W, DIM = 8, 32, 32, 512
HEADS, DH = 8, 64
M = H * W          # 1024 query/key positions
NT = M // 128      # 8 n-tiles / m-tiles / pos-tiles

_CACHE = {}


def _build(loop_n=1, parts="all", e_bf16=True, probe=0, rsb=False,
           pairgrp=True, mcmerge=True, **_flags):
    import concourse.bass as bass
    import concourse.mybir as mybir
    import concourse.tile as tile
    from concourse import bacc
    from contextlib import ExitStack

    F32 = mybir.dt.float32
    F32R = mybir.dt.float32r
    BF16 = mybir.dt.bfloat16
    EXP = mybir.ActivationFunctionType.Exp
    COPY = mybir.ActivationFunctionType.Copy

    nc = bacc.Bacc("TRN2", target_bir_lowering=False, debug=False,
                   enable_asserts=False, num_devices=8)
    x = nc.dram_tensor("x", [M, DIM], BF16, kind="ExternalInput").ap()
    w_qkv = nc.dram_tensor("w_qkv", [DIM, 3 * DIM], BF16, kind="ExternalInput").ap()
    w_out = nc.dram_tensor("w_out", [DIM, DIM], BF16, kind="ExternalInput").ap()
    ident = nc.dram_tensor("ident", [128, 128], BF16, kind="ExternalInput").ap()
    i4big = nc.dram_tensor("i4big", [128, 512], BF16, kind="ExternalInput").ap()
    i4big32 = nc.dram_tensor("i4big32", [128, 512], F32, kind="ExternalInput").ap()
    sel = nc.dram_tensor("sel", [128, 128], BF16, kind="ExternalInput").ap()
    y = nc.dram_tensor("y", [DIM, M], F32, kind="ExternalOutput").ap()

    with tile.TileContext(nc) as tc, ExitStack() as top:
        if loop_n > 1:
            top.enter_context(tc.For_i(0, loop_n, 1))
        persist = top.enter_context(tc.tile_pool(name="persist", bufs=1))

        # ---- persistent constants ----
        ident_sb = persist.tile([128, 128], BF16, tag="ident")
        nc.sync.dma_start(out=ident_sb, in_=ident)
        if e_bf16:
            i4_sb = persist.tile([128, 512], BF16, tag="i4")
            nc.sync.dma_start(out=i4_sb, in_=i4big)
        else:
            i4_sb = persist.tile([128, 512], F32R, tag="i4")
            nc.sync.dma_start(out=i4_sb, in_=i4big32.bitcast(F32R))
        sel_sb = persist.tile([128, 128], BF16, tag="sel")
        nc.sync.dma_start(out=sel_sb, in_=sel)
        wo_sb = [persist.tile([128, DIM], BF16, tag=f"wo{i}", name=f"wo{i}") for i in range(4)]
        for i in range(4):
            nc.sync.dma_start(out=wo_sb[i], in_=w_out[128 * i:128 * (i + 1), :])

        # ---- persistent activations ----
        EDT = BF16 if e_bf16 else F32R
        probe_ones = None
        if probe:
            probe_ones_m = persist.tile([128, M], BF16, tag="probe1")
            nc.vector.memset(probe_ones_m, 1.0)
            probe_ones = probe_ones_m[:, :512]
        qkT_sb = [persist.tile([128, M], BF16, tag=f"qkT{i}", name=f"qkT{i}") for i in range(8)]
        v_sb = [persist.tile([128, DIM], EDT, tag=f"v{i}", name=f"v{i}") for i in range(NT)]
        outT_sb = [persist.tile([128, M], BF16, tag=f"outT{i}", name=f"outT{i}") for i in range(4)]

        # ================= FRONT =================
        with ExitStack() as fr:
            fsb = fr.enter_context(tc.tile_pool(name="front_sb", bufs=1))
            xt_pool = fr.enter_context(tc.tile_pool(name="xt_ps", bufs=2, space="PSUM"))
            mm_pool = fr.enter_context(tc.tile_pool(name="mm_ps", bufs=2, space="PSUM"))
            vp_pool = fr.enter_context(tc.tile_pool(name="vp_ps", bufs=2, space="PSUM"))

            x_sb = [fsb.tile([128, DIM], BF16, tag=f"x{mt}", name=f"x{mt}") for mt in range(NT)]
            for mt in range(NT):
                nc.sync.dma_start(out=x_sb[mt], in_=x[128 * mt:128 * (mt + 1), :])
            wq_sb = [fsb.tile([128, 3 * DIM], BF16, tag=f"wq{kt}", name=f"wq{kt}") for kt in range(4)]
            for kt in range(4):
                nc.sync.dma_start(out=wq_sb[kt], in_=w_qkv[128 * kt:128 * (kt + 1), :])

            # x transpose: xT[kc] [128 k, 1024 m]
            xT_sb = []
            for kc in range(4):
                xt_ps = xt_pool.tile([128, M], BF16, tag="xt")
                for mt in range(NT):
                    nc.tensor.matmul(xt_ps[:, 128 * mt:128 * (mt + 1)],
                                     x_sb[mt][:, 128 * kc:128 * (kc + 1)],
                                     ident_sb, is_transpose=True,
                                     start=True, stop=True)
                t = fsb.tile([128, M], BF16, tag=f"xT{kc}", name=f"xT{kc}")
                nc.scalar.activation(out=t, in_=xt_ps, func=COPY)
                xT_sb.append(t)

            # qkT[ft] = (x @ w_qkv[:, :1024]).T f-tile ft
            # heads (2s, 2s+1) need ft s (q) and 4+s (k): emit s-pair order
            for ft in (0, 4, 1, 5, 2, 6, 3, 7):
                qk_ps = mm_pool.tile([128, M], F32, tag="mm")
                for mc in range(2):
                    for kt in range(4):
                        nc.tensor.matmul(
                            qk_ps[:, 512 * mc:512 * (mc + 1)],
                            wq_sb[kt][:, 128 * ft:128 * (ft + 1)],
                            xT_sb[kt][:, 512 * mc:512 * (mc + 1)],
                            start=(kt == 0), stop=(kt == 3))
                nc.scalar.activation(out=qkT_sb[ft], in_=qk_ps, func=COPY)

            # v natural: v[pt] [128 pos, 512 vf]
            for pt in range(NT):
                v_ps = vp_pool.tile([128, DIM], F32, tag="vp")
                for kt in range(4):
                    nc.tensor.matmul(v_ps,
                                     xT_sb[kt][:, 128 * pt:128 * (pt + 1)],
                                     wq_sb[kt][:, 1024:1536],
                                     start=(kt == 0), stop=(kt == 3))
                nc.scalar.activation(out=v_sb[pt], in_=v_ps, func=COPY)

        # ================= ATTENTION =================
        with ExitStack() as at:
            e_pool = at.enter_context(tc.tile_pool(name="e_sb", bufs=24))
            strip_pool = at.enter_context(tc.tile_pool(name="strip_sb", bufs=2))
            rsb_pool = at.enter_context(tc.tile_pool(name="rsb_sb", bufs=8)) \
                if rsb else None
            if mcmerge:
                s_pool = at.enter_context(tc.tile_pool(name="s_ps", bufs=2, space="PSUM"))
                zr_pool = at.enter_context(tc.tile_pool(name="zr_ps", bufs=2, space="PSUM"))
            else:
                s_pool = at.enter_context(tc.tile_pool(name="s_ps", bufs=2, space="PSUM"))
                zr_pool = at.enter_context(tc.tile_pool(name="zr_ps", bufs=2, space="PSUM"))
            pv_pool = at.enter_context(tc.tile_pool(name="pv_ps", bufs=1, space="PSUM"))

            E = {}       # (s, hh, nt) -> tile
            pv_tiles = {}

            def alpha_chunks(s):
                out = []
                for nt in range(NT):
                    for hh in range(2):
                        def emit(s=s, nt=nt, hh=hh):
                            h = 2 * s + hh
                            off = 64 * hh
                            qt = qkT_sb[h // 2]
                            kt_ = qkT_sb[4 + h // 2]
                            e = e_pool.tile([128, M], EDT, tag="E",
                                            name=f"E_{s}_{nt}_{hh}")
                            if mcmerge:
                                for mc in range(2):
                                    s_ps = s_pool.tile(
                                        [128, 512], F32, tag="s",
                                        name=f"sps_{s}_{nt}_{hh}_{mc}")
                                    nc.tensor.matmul(
                                        s_ps,
                                        kt_[off:off + 64, 128 * nt:128 * (nt + 1)],
                                        qt[off:off + 64, 512 * mc:512 * (mc + 1)],
                                        start=True, stop=True,
                                        tile_position=(off, 0))
                                    nc.scalar.activation(
                                        out=e[:, 512 * mc:512 * (mc + 1)],
                                        in_=s_ps, func=EXP, scale=0.125)
                            else:
                                s_ps = s_pool.tile([128, M], F32, tag="s",
                                                   name=f"sps_{s}_{nt}_{hh}")
                                for mc in range(2):
                                    nc.tensor.matmul(
                                        s_ps[:, 512 * mc:512 * (mc + 1)],
                                        kt_[off:off + 64, 128 * nt:128 * (nt + 1)],
                                        qt[off:off + 64, 512 * mc:512 * (mc + 1)],
                                        start=True, stop=True,
                                        tile_position=(off, 0))
                                nc.scalar.activation(out=e, in_=s_ps, func=EXP,
                                                     scale=0.125)
                            E[s, hh, nt] = e
                        out.append(emit)
                return out

            def beta_chunks_mc(s):
                """mc-merged: 2 units (hh); z/R/mult at [128,1024]; pv lags
                sel by one step."""
                chunks = []
                strips = {}

                def mkpv(s=s):
                    pv_tiles[s] = pv_pool.tile([128, M], F32, tag="pv",
                                               name=f"pv_{s}")
                chunks.append(mkpv)

                def zrc(hh, s=s):
                    zt = {}
                    for mc in range(2):
                        zt[mc] = zr_pool.tile([128, M], F32, tag="zr",
                                              name=f"z_{s}_{hh}_{mc}")
                    # 4 consecutive z matmuls share each i4 slice
                    for nt in (0, 4, 1, 5, 2, 6, 3, 7):
                        r, a = nt // 4, nt % 4
                        for mc in range(2):
                            nc.tensor.matmul(
                                zt[mc][:, 512 * r:512 * (r + 1)],
                                i4_sb[:, 128 * a:128 * (a + 1)],
                                E[s, hh, nt][:, 512 * mc:512 * (mc + 1)],
                                start=(a == 0), stop=(a == 3))
                    for mc in range(2):
                        stf = strip_pool.tile([128, M], F32, tag="stripf",
                                              name=f"stripf_{s}_{hh}_{mc}")
                        nc.vector.reciprocal_approx_fast(out=stf, in_=zt[mc])
                        st = strip_pool.tile([128, M], BF16, tag="strip",
                                             name=f"strip_{s}_{hh}_{mc}")
                        nc.scalar.activation(out=st, in_=stf, func=COPY)
                        strips[hh, mc] = st

                def sel_one(hh, nt, s=s):
                    r, a = nt // 4, nt % 4
                    r_ps = zr_pool.tile([128, M], F32, tag="zr",
                                        name=f"rps_{s}_{hh}_{nt}")
                    for mc in range(2):
                        nc.tensor.matmul(
                            r_ps[:, 512 * mc:512 * (mc + 1)],
                            sel_sb[32 * a:32 * a + 4, :],
                            strips[hh, mc][32 * a:32 * a + 4,
                                           512 * r:512 * (r + 1)],
                            start=True, stop=True,
                            tile_position=(32 * a, 0))
                    Rm[hh, nt] = r_ps

                def mult_pv(hh, nt, s=s):
                    h = 2 * s + hh
                    nc.vector.tensor_mul(
                        out=E[s, hh, nt], in0=E[s, hh, nt],
                        in1=probe_ones_m if probe else Rm[hh, nt])
                    for mc in range(2):
                        mcs = slice(512 * mc, 512 * (mc + 1))
                        nc.tensor.matmul(
                            pv_tiles[s][64 * hh:64 * (hh + 1), mcs],
                            v_sb[nt][:, 64 * h:64 * (h + 1)],
                            E[s, hh, nt][:, mcs],
                            start=(nt == 0), stop=(nt == 7))

                Rm = {}
                ntseq = (0, 4, 1, 5, 2, 6, 3, 7)
                for hh in range(2):
                    chunks.append(lambda hh=hh: zrc(hh))
                    for i, nt in enumerate(ntseq):
                        chunks.append(lambda hh=hh, nt=nt: sel_one(hh, nt))
                        if i >= 1:
                            chunks.append(
                                lambda hh=hh, p=ntseq[i - 1]: mult_pv(hh, p))
                    chunks.append(lambda hh=hh: mult_pv(hh, ntseq[7]))

                def copyout(s=s):
                    nc.scalar.activation(out=outT_sb[s], in_=pv_tiles[s],
                                         func=COPY)
                chunks.append(copyout)
                return chunks

            def beta_chunks(s):
                """Per head-pair s: 4 units (hh, mc), each: Z group-sums ->
                reciprocal -> bf16 cast -> per nt: R broadcast (PE), E*R
                (DVE), PV accumulate (PE). Software-pipelined: pv lags
                sel/mult by one step so PE never head-of-line blocks on the
                DVE multiply; the next unit's Z chain is emitted early."""
                chunks = []
                units = [(hh, mc) for hh in range(2) for mc in range(2)]
                strips = {}

                def mkpv(s=s):
                    pv_tiles[s] = pv_pool.tile([128, M], F32, tag="pv",
                                               name=f"pv_{s}")
                chunks.append(mkpv)

                def zrc(u, s=s):
                    hh, mc = units[u]
                    mcs = slice(512 * mc, 512 * (mc + 1))
                    zps = [zr_pool.tile([128, 512], F32, tag="zr",
                                        name=f"z_{s}_{hh}_{mc}_{r}")
                           for r in range(2)]
                    # pair order: consecutive matmuls share the i4 stationary
                    for nt in (0, 4, 1, 5, 2, 6, 3, 7):
                        r, a = nt // 4, nt % 4
                        nc.tensor.matmul(
                            zps[r],
                            i4_sb[:, 128 * a:128 * (a + 1)],
                            E[s, hh, nt][:, mcs],
                            start=(a == 0), stop=(a == 3))
                    stf = strip_pool.tile([128, M], F32, tag="stripf",
                                          name=f"stripf_{s}_{hh}_{mc}")
                    for r in range(2):
                        nc.vector.reciprocal_approx_fast(
                            out=stf[:, 512 * r:512 * (r + 1)], in_=zps[r])
                    st = strip_pool.tile([128, M], BF16, tag="strip",
                                         name=f"strip_{s}_{hh}_{mc}")
                    nc.scalar.activation(out=st, in_=stf, func=COPY)
                    strips[u] = st

                Rsb = {}

                def sel_pair(u, n0, n1, s=s):
                    """R broadcast: sel matmul -> PSUM, ACT copy -> SBUF bf16."""
                    hh, mc = units[u]
                    for nt in (n0, n1):
                        r, a = nt // 4, nt % 4
                        if probe >= 2:
                            continue
                        r_ps = zr_pool.tile([128, 512], F32, tag="zr",
                                            name=f"rps_{s}_{hh}_{mc}_{nt}")
                        nc.tensor.matmul(
                            r_ps, sel_sb[32 * a:32 * a + 4, :],
                            strips[u][32 * a:32 * a + 4, 512 * r:512 * (r + 1)],
                            start=True, stop=True,
                            tile_position=(32 * a, 0))
                        if rsb:
                            t = rsb_pool.tile([128, 512], BF16, tag="rsb",
                                              name=f"rsb_{s}_{hh}_{mc}_{nt}")
                            nc.scalar.activation(out=t, in_=r_ps, func=COPY)
                            Rsb[u, nt] = t
                        else:
                            Rsb[u, nt] = r_ps

                def mult_pv_pair(u, n0, n1, s=s):
                    hh, mc = units[u]
                    h = 2 * s + hh
                    mcs = slice(512 * mc, 512 * (mc + 1))
                    for nt in (n0, n1):
                        nc.vector.tensor_mul(
                            out=E[s, hh, nt][:, mcs],
                            in0=E[s, hh, nt][:, mcs],
                            in1=probe_ones if probe else Rsb[u, nt])
                    for nt in (n0, n1):
                        nc.tensor.matmul(
                            pv_tiles[s][64 * hh:64 * (hh + 1), mcs],
                            v_sb[nt][:, 64 * h:64 * (h + 1)],
                            E[s, hh, nt][:, mcs],
                            start=(nt == 0), stop=(nt == 7))

                def sel_one(u, nt, s=s):
                    hh, mc = units[u]
                    r, a = nt // 4, nt % 4
                    if probe >= 2:
                        return
                    r_ps = zr_pool.tile([128, 512], F32, tag="zr",
                                        name=f"rps_{s}_{hh}_{mc}_{nt}")
                    nc.tensor.matmul(
                        r_ps, sel_sb[32 * a:32 * a + 4, :],
                        strips[u][32 * a:32 * a + 4, 512 * r:512 * (r + 1)],
                        start=True, stop=True,
                        tile_position=(32 * a, 0))
                    Rsb[u, nt] = r_ps

                def mult_pv_one(u, nt, s=s):
                    hh, mc = units[u]
                    h = 2 * s + hh
                    mcs = slice(512 * mc, 512 * (mc + 1))
                    nc.vector.tensor_mul(
                        out=E[s, hh, nt][:, mcs],
                        in0=E[s, hh, nt][:, mcs],
                        in1=probe_ones if probe else Rsb[u, nt])
                    nc.tensor.matmul(
                        pv_tiles[s][64 * hh:64 * (hh + 1), mcs],
                        v_sb[nt][:, 64 * h:64 * (h + 1)],
                        E[s, hh, nt][:, mcs],
                        start=(nt == 0), stop=(nt == 7))

                if pairgrp:
                    pairs = [(0, 4), (1, 5), (2, 6), (3, 7)]
                    for u in range(4):
                        chunks.append(lambda u=u: zrc(u))
                        for pi, (n0, n1) in enumerate(pairs):
                            chunks.append(
                                lambda u=u, n0=n0, n1=n1: sel_pair(u, n0, n1))
                            if pi >= 1:
                                p0, p1 = pairs[pi - 1]
                                chunks.append(
                                    lambda u=u, p0=p0, p1=p1: mult_pv_pair(u, p0, p1))
                        chunks.append(
                            lambda u=u: mult_pv_pair(u, pairs[3][0], pairs[3][1]))
                else:
                    ntseq = (0, 4, 1, 5, 2, 6, 3, 7)
                    for u in range(4):
                        chunks.append(lambda u=u: zrc(u))
                        for i, nt in enumerate(ntseq):
                            chunks.append(lambda u=u, nt=nt: sel_one(u, nt))
                            if i >= 1:
                                chunks.append(
                                    lambda u=u, p=ntseq[i - 1]: mult_pv_one(u, p))
                        chunks.append(lambda u=u: mult_pv_one(u, ntseq[7]))

                def copyout(s=s):
                    nc.scalar.activation(out=outT_sb[s], in_=pv_tiles[s],
                                         func=COPY)
                chunks.append(copyout)
                return chunks

            # software-pipeline: alpha(s) interleaved with beta(s-1)
            npair = 4
            for s in range(npair + 1):
                a = alpha_chunks(s) if s < npair and parts != "front" else []
                mk_beta = beta_chunks_mc if mcmerge else beta_chunks
                b = mk_beta(s - 1) if s >= 1 and parts in ("fab", "all") else []
                na, nb = len(a), len(b)
                if not a:
                    for f in b:
                        f()
                else:
                    ratio = nb / na if na else 0
                    bi = 0.0
                    for i, f in enumerate(a):
                        f()
                        target = (i + 1) * ratio
                        while bi < target and int(bi) < nb:
                            b[int(bi)]()
                            bi += 1
                    for j in range(int(bi), nb):
                        b[j]()

        # ================= PROJ (yT = w_out.T-contract @ outT) =================
        if parts != "all":
            # keep y written so the output tensor exists
            with ExitStack() as pj:
                y_pool = pj.enter_context(tc.tile_pool(name="y0_sb", bufs=1))
                y_sb = y_pool.tile([128, M], F32, tag="y0")
                nc.vector.memset(y_sb, 0.0)
                for dc in range(4):
                    nc.sync.dma_start(out=y[128 * dc:128 * (dc + 1), :], in_=y_sb)
        elif True:
          with ExitStack() as pj:
            pj_pool = pj.enter_context(tc.tile_pool(name="pj_ps", bufs=2, space="PSUM"))
            y_pool = pj.enter_context(tc.tile_pool(name="y_sb", bufs=2))
            for dc in range(4):
                y_sb = y_pool.tile([128, M], F32, tag="y")
                for mc in range(2):
                    p = pj_pool.tile([128, 512], F32, tag="pj")
                    for kt in range(4):
                        nc.tensor.matmul(p,
                                         wo_sb[kt][:, 128 * dc:128 * (dc + 1)],
                                         outT_sb[kt][:, 512 * mc:512 * (mc + 1)],
                                         start=(kt == 0), stop=(kt == 3))
                    nc.scalar.activation(out=y_sb[:, 512 * mc:512 * (mc + 1)],
                                         in_=p, func=COPY)
                nc.sync.dma_start(out=y[128 * dc:128 * (dc + 1), :], in_=y_sb)

    nc.compile()
    return nc


def _consts():
    import ml_dtypes
    bf16 = ml_dtypes.bfloat16
    ident = np.eye(128, dtype=np.float32).astype(bf16)
    # i4big[:, 128a:128(a+1)] maps E-tile partition p -> strip row 32a + p//32
    i4big = np.zeros((128, 512), np.float32)
    for a in range(4):
        for p in range(128):
            i4big[p, 128 * a + 32 * a + p // 32] = 1.0
    i4big_bf = i4big.astype(bf16)
    sel = np.zeros((128, 128), np.float32)
    for p in range(128):
        if p % 32 < 4:
            for c in range(128):
                if c // 32 == p % 32:
                    sel[p, c] = 1.0
    return ident, i4big_bf, sel.astype(bf16)


def _in_maps(x, w_qkv, w_out):
    import ml_dtypes
    bf16 = ml_dtypes.bfloat16
    ident, i4big, sel = _consts()
    x = np.asarray(x, dtype=np.float32)
    wq = np.asarray(w_qkv, np.float32).astype(bf16)
    wo = np.asarray(w_out, np.float32).astype(bf16)
    maps = []
    for c in range(8):
        maps.append({
            "x": np.ascontiguousarray(x[c].reshape(M, DIM)).astype(bf16),
            "w_qkv": wq, "w_out": wo,
            "ident": ident, "i4big": i4big,
            "i4big32": i4big.astype(np.float32), "sel": sel,
        })
    return maps


def kernel(x, w_qkv, w_out, b_out):
    from concourse import bass_utils
    if "nc" not in _CACHE:
        _CACHE["nc"] = _build()
    nc = _CACHE["nc"]
    in_maps = _in_maps(x, w_qkv, w_out)
    res = bass_utils.run_bass_kernel_spmd(nc, in_maps, core_ids=list(range(8)))
    b = np.asarray(b_out, np.float32).reshape(1, DIM)
    out = np.stack([
        (res.results[c]["y"].T + b).reshape(H, W, DIM) for c in range(8)
    ])
    return out


# revision 54
# speedup vs baseline: 3.1094x; 1.5548x over previous
"""AxialAttention TRN2 Bass kernel — 8-core data-parallel over batch (v2, bf16).

Reference math (per batch element b, per head h):
  qkv = x @ w_qkv;  q,k,v split; heads of dh=64
  S[m, n] = q_m . k_n / 8   (m, n over 1024 = 32x32 positions)
  attn = softmax over y only, where n = x*32 + y  (groups of 32 consecutive n)
  out[m] = sum_n attn[m, n] v[n];  y = out @ w_out + b_out

v2 changes vs v1:
  - bf16 operands everywhere on the matmul path (host converts inputs);
    PSUM accumulation stays f32.
  - Z group-sums matmul directly into sel-strip layout (i4big weights):
    no rz relocation DMAs, no rz dtype-copy (bitcast f32->f32r instead).
  - pv accumulates all 4 (hh, mc) quadrants of a head-pair in one
    [128,1024] PSUM tile; single ACT copyout -> outT (no stage merges).
  - bias b_out applied on host; y emitted transposed [DIM, M] and
    transposed back on host.
  - front PSUM->SBUF copies on ACT (DVE reserved for attention mults).

Per-core layout:
  xT   4x[128 k, 1024 m] bf16 (PE transpose of x)
  qkT  8x[128 f, 1024 m] bf16 = (x @ w_qkv[:, :1024]).T
  v    8x[128 pos, 512 vf] bf16
  E^T  per (s, hh, nt): [128 n, 1024 m] bf16 = exp(S^T/8)
  Z    strips [128, 512] f32 per (hh, mc, r): group sums in sel-strip rows
  R    [128 n, 512 m] f32 PSUM via sel matmuls; E' = E * R (DVE)
  outT 4x[128 (2 heads x dh), 1024 m] bf16
  yT   [512 dim, 1024 m] f32 -> DRAM; host transposes + adds bias
"""
import numpy as np

B, H, W, DIM = 8, 32, 32, 512
HEADS, DH = 8, 64
M = H * W          # 1024 query/key positions
NT = M // 128      # 8 n-tiles / m-tiles / pos-tiles

_CACHE = {}


def _build(loop_n=1, parts="all", e_bf16=True, probe=0, rsb=False,
           pairgrp=True, mcmerge=True, **_flags):
    import concourse.bass as bass
    import concourse.mybir as mybir
    import concourse.tile as tile
    from concourse import bacc
    from contextlib import ExitStack

    F32 = mybir.dt.float32
    F32R = mybir.dt.float32r
    BF16 = mybir.dt.bfloat16
    EXP = mybir.ActivationFunctionType.Exp
    COPY = mybir.ActivationFunctionType.Copy

    nc = bacc.Bacc("TRN2", target_bir_lowering=False, debug=False,
                   enable_asserts=False, num_devices=8)
    x = nc.dram_tensor("x", [M, DIM], BF16, kind="ExternalInput").ap()
    w_qkv = nc.dram_tensor("w_qkv", [DIM, 3 * DIM], BF16, kind="ExternalInput").ap()
    w_out = nc.dram_tensor("w_out", [DIM, DIM], BF16, kind="ExternalInput").ap()
    ident = nc.dram_tensor("ident", [128, 128], BF16, kind="ExternalInput").ap()
    i4big = nc.dram_tensor("i4big", [128, 512], BF16, kind="ExternalInput").ap()
    i4big32 = nc.dram_tensor("i4big32", [128, 512], F32, kind="ExternalInput").ap()
    sel = nc.dram_tensor("sel", [128, 128], BF16, kind="ExternalInput").ap()
    y = nc.dram_tensor("y", [DIM, M], BF16, kind="ExternalOutput").ap()

    with tile.TileContext(nc) as tc, ExitStack() as top:
        if loop_n > 1:
            top.enter_context(tc.For_i(0, loop_n, 1))
        persist = top.enter_context(tc.tile_pool(name="persist", bufs=1))

        # ---- persistent constants ----
        ident_sb = persist.tile([128, 128], BF16, tag="ident")
        nc.sync.dma_start(out=ident_sb, in_=ident)
        if e_bf16:
            i4_sb = persist.tile([128, 512], BF16, tag="i4")
            nc.sync.dma_start(out=i4_sb, in_=i4big)
        else:
            i4_sb = persist.tile([128, 512], F32R, tag="i4")
            nc.sync.dma_start(out=i4_sb, in_=i4big32.bitcast(F32R))
        sel_sb = persist.tile([128, 128], BF16, tag="sel")
        nc.sync.dma_start(out=sel_sb, in_=sel)
        wo_sb = [persist.tile([128, DIM], BF16, tag=f"wo{i}", name=f"wo{i}") for i in range(4)]
        for i in range(4):
            nc.sync.dma_start(out=wo_sb[i], in_=w_out[128 * i:128 * (i + 1), :])

        # ---- persistent activations ----
        EDT = BF16 if e_bf16 else F32R
        probe_ones = None
        if probe:
            probe_ones_m = persist.tile([128, M], BF16, tag="probe1")
            nc.vector.memset(probe_ones_m, 1.0)
            probe_ones = probe_ones_m[:, :512]
        qkT_sb = [persist.tile([128, M], BF16, tag=f"qkT{i}", name=f"qkT{i}") for i in range(8)]
        v_sb = [persist.tile([128, DIM], EDT, tag=f"v{i}", name=f"v{i}") for i in range(NT)]
        outT_sb = [persist.tile([128, M], BF16, tag=f"outT{i}", name=f"outT{i}") for i in range(4)]

        # ============ FRONT (fused into the attention pipeline) ============
        with ExitStack() as at:
            e_pool = at.enter_context(tc.tile_pool(name="e_sb", bufs=24))
            strip_pool = at.enter_context(tc.tile_pool(name="strip_sb", bufs=2))
            rsb_pool = at.enter_context(tc.tile_pool(name="rsb_sb", bufs=8)) \
                if rsb else None
            if mcmerge:
                s_pool = at.enter_context(tc.tile_pool(name="s_ps", bufs=2, space="PSUM"))
                zr_pool = at.enter_context(tc.tile_pool(name="zr_ps", bufs=2, space="PSUM"))
            else:
                s_pool = at.enter_context(tc.tile_pool(name="s_ps", bufs=2, space="PSUM"))
                zr_pool = at.enter_context(tc.tile_pool(name="zr_ps", bufs=2, space="PSUM"))
            pv_pool = at.enter_context(tc.tile_pool(name="pv_ps", bufs=1, space="PSUM"))

            E = {}       # (s, hh, nt) -> tile
            pv_tiles = {}

            # ---- front: x/w loads, xT transposes, qkT / v projections ----
            assert mcmerge, "fused front requires the mcmerge pipeline"
            fsb = at.enter_context(tc.tile_pool(name="front_sb", bufs=1))
            x_sb = [fsb.tile([128, DIM], BF16, tag=f"x{mt}", name=f"x{mt}")
                    for mt in range(NT)]
            for mt in range(NT):
                nc.sync.dma_start(out=x_sb[mt], in_=x[128 * mt:128 * (mt + 1), :])
            wq_sb = [fsb.tile([128, 3 * DIM], BF16, tag=f"wq{kt}", name=f"wq{kt}")
                     for kt in range(4)]
            # split by q/k/v column groups so the first qkT matmuls wait on a
            # 128KB transfer, not a 384KB one; spread across DMA queues
            for grp in range(3):
                gs = slice(512 * grp, 512 * (grp + 1))
                for kt in range(4):
                    nc.sync.dma_start(out=wq_sb[kt][:, gs],
                                      in_=w_qkv[128 * kt:128 * (kt + 1), gs])
            xT_sb = [fsb.tile([128, M], BF16, tag=f"xT{kc}", name=f"xT{kc}")
                     for kc in range(4)]

            def fr_xt(kc):
                # transpose via a bf16 bitcast view of a [128,512]-f32 ring tile
                xt_ps = s_pool.tile([128, 512], F32, tag="s",
                                    name=f"xt_{kc}").bitcast(BF16)
                for mt in range(NT):
                    nc.tensor.matmul(xt_ps[:, 128 * mt:128 * (mt + 1)],
                                     x_sb[mt][:, 128 * kc:128 * (kc + 1)],
                                     ident_sb, is_transpose=True,
                                     start=True, stop=True)
                nc.scalar.activation(out=xT_sb[kc], in_=xt_ps, func=COPY)

            def fr_qkT(ft):
                qk_ps = zr_pool.tile([128, M], F32, tag="zr", name=f"qk_{ft}")
                for mc in range(2):
                    for kt in range(4):
                        nc.tensor.matmul(
                            qk_ps[:, 512 * mc:512 * (mc + 1)],
                            wq_sb[kt][:, 128 * ft:128 * (ft + 1)],
                            xT_sb[kt][:, 512 * mc:512 * (mc + 1)],
                            start=(kt == 0), stop=(kt == 3))
                nc.scalar.activation(out=qkT_sb[ft], in_=qk_ps, func=COPY)

            def fr_v(pt):
                v_ps = s_pool.tile([128, DIM], F32, tag="s", name=f"vp_{pt}")
                for kt in range(4):
                    nc.tensor.matmul(v_ps,
                                     xT_sb[kt][:, 128 * pt:128 * (pt + 1)],
                                     wq_sb[kt][:, 1024:1536],
                                     start=(kt == 0), stop=(kt == 3))
                nc.scalar.activation(out=v_sb[pt], in_=v_ps, func=COPY)

            # emitted immediately: transposes + the qkT tiles alpha(0) needs
            for kc in range(4):
                fr_xt(kc)
            fr_qkT(0)
            fr_qkT(4)
            # remaining front work is interleaved with alpha(0) below
            front_rest = [lambda f=ft: fr_qkT(f) for ft in (1, 5)]
            front_rest += [lambda p=pt: fr_v(p) for pt in range(NT)]
            front_rest += [lambda f=ft: fr_qkT(f) for ft in (2, 6, 3, 7)]

            def alpha_chunks(s):
                out = []
                for nt in range(NT):
                    for hh in range(2):
                        def emit(s=s, nt=nt, hh=hh):
                            h = 2 * s + hh
                            off = 64 * hh
                            qt = qkT_sb[h // 2]
                            kt_ = qkT_sb[4 + h // 2]
                            e = e_pool.tile([128, M], EDT, tag="E",
                                            name=f"E_{s}_{nt}_{hh}")
                            if mcmerge:
                                for mc in range(2):
                                    s_ps = s_pool.tile(
                                        [128, 512], F32, tag="s",
                                        name=f"sps_{s}_{nt}_{hh}_{mc}")
                                    nc.tensor.matmul(
                                        s_ps,
                                        kt_[off:off + 64, 128 * nt:128 * (nt + 1)],
                                        qt[off:off + 64, 512 * mc:512 * (mc + 1)],
                                        start=True, stop=True,
                                        tile_position=(off, 0))
                                    nc.scalar.activation(
                                        out=e[:, 512 * mc:512 * (mc + 1)],
                                        in_=s_ps, func=EXP, scale=0.125)
                            else:
                                s_ps = s_pool.tile([128, M], F32, tag="s",
                                                   name=f"sps_{s}_{nt}_{hh}")
                                for mc in range(2):
                                    nc.tensor.matmul(
                                        s_ps[:, 512 * mc:512 * (mc + 1)],
                                        kt_[off:off + 64, 128 * nt:128 * (nt + 1)],
                                        qt[off:off + 64, 512 * mc:512 * (mc + 1)],
                                        start=True, stop=True,
                                        tile_position=(off, 0))
                                nc.scalar.activation(out=e, in_=s_ps, func=EXP,
                                                     scale=0.125)
                            E[s, hh, nt] = e
                        out.append(emit)
                return out

            def beta_chunks_mc(s):
                """mc-merged: 2 units (hh); z/R/mult at [128,1024]; pv lags
                sel by one step."""
                chunks = []
                strips = {}

                def mkpv(s=s):
                    pv_tiles[s] = pv_pool.tile([128, M], F32, tag="pv",
                                               name=f"pv_{s}")
                chunks.append(mkpv)

                def zrc(hh, s=s):
                    zt = {}
                    for mc in range(2):
                        zt[mc] = zr_pool.tile([128, M], F32, tag="zr",
                                              name=f"z_{s}_{hh}_{mc}")
                    # 4 consecutive z matmuls share each i4 slice
                    for nt in (0, 4, 1, 5, 2, 6, 3, 7):
                        r, a = nt // 4, nt % 4
                        for mc in range(2):
                            nc.tensor.matmul(
                                zt[mc][:, 512 * r:512 * (r + 1)],
                                i4_sb[:, 128 * a:128 * (a + 1)],
                                E[s, hh, nt][:, 512 * mc:512 * (mc + 1)],
                                start=(a == 0), stop=(a == 3))
                    for mc in range(2):
                        stf = strip_pool.tile([128, M], F32, tag="stripf",
                                              name=f"stripf_{s}_{hh}_{mc}")
                        nc.vector.reciprocal_approx_fast(out=stf, in_=zt[mc])
                        st = strip_pool.tile([128, M], BF16, tag="strip",
                                             name=f"strip_{s}_{hh}_{mc}")
                        nc.scalar.activation(out=st, in_=stf, func=COPY)
                        strips[hh, mc] = st

                def sel_one(hh, nt, s=s):
                    r, a = nt // 4, nt % 4
                    r_ps = zr_pool.tile([128, M], F32, tag="zr",
                                        name=f"rps_{s}_{hh}_{nt}")
                    for mc in range(2):
                        nc.tensor.matmul(
                            r_ps[:, 512 * mc:512 * (mc + 1)],
                            sel_sb[32 * a:32 * a + 4, :],
                            strips[hh, mc][32 * a:32 * a + 4,
                                           512 * r:512 * (r + 1)],
                            start=True, stop=True,
                            tile_position=(32 * a, 0))
                    Rm[hh, nt] = r_ps

                def mult_pv(hh, nt, s=s):
                    h = 2 * s + hh
                    nc.vector.tensor_mul(
                        out=E[s, hh, nt], in0=E[s, hh, nt],
                        in1=probe_ones_m if probe else Rm[hh, nt])
                    for mc in range(2):
                        mcs = slice(512 * mc, 512 * (mc + 1))
                        nc.tensor.matmul(
                            pv_tiles[s][64 * hh:64 * (hh + 1), mcs],
                            v_sb[nt][:, 64 * h:64 * (h + 1)],
                            E[s, hh, nt][:, mcs],
                            start=(nt == 0), stop=(nt == 7))

                Rm = {}
                ntseq = (0, 4, 1, 5, 2, 6, 3, 7)
                for hh in range(2):
                    chunks.append(lambda hh=hh: zrc(hh))
                    for i, nt in enumerate(ntseq):
                        chunks.append(lambda hh=hh, nt=nt: sel_one(hh, nt))
                        if i >= 1:
                            chunks.append(
                                lambda hh=hh, p=ntseq[i - 1]: mult_pv(hh, p))
                    chunks.append(lambda hh=hh: mult_pv(hh, ntseq[7]))

                def copyout(s=s):
                    nc.scalar.activation(out=outT_sb[s], in_=pv_tiles[s],
                                         func=COPY)
                chunks.append(copyout)
                return chunks

            def beta_chunks(s):
                """Per head-pair s: 4 units (hh, mc), each: Z group-sums ->
                reciprocal -> bf16 cast -> per nt: R broadcast (PE), E*R
                (DVE), PV accumulate (PE). Software-pipelined: pv lags
                sel/mult by one step so PE never head-of-line blocks on the
                DVE multiply; the next unit's Z chain is emitted early."""
                chunks = []
                units = [(hh, mc) for hh in range(2) for mc in range(2)]
                strips = {}

                def mkpv(s=s):
                    pv_tiles[s] = pv_pool.tile([128, M], F32, tag="pv",
                                               name=f"pv_{s}")
                chunks.append(mkpv)

                def zrc(u, s=s):
                    hh, mc = units[u]
                    mcs = slice(512 * mc, 512 * (mc + 1))
                    zps = [zr_pool.tile([128, 512], F32, tag="zr",
                                        name=f"z_{s}_{hh}_{mc}_{r}")
                           for r in range(2)]
                    # pair order: consecutive matmuls share the i4 stationary
                    for nt in (0, 4, 1, 5, 2, 6, 3, 7):
                        r, a = nt // 4, nt % 4
                        nc.tensor.matmul(
                            zps[r],
                            i4_sb[:, 128 * a:128 * (a + 1)],
                            E[s, hh, nt][:, mcs],
                            start=(a == 0), stop=(a == 3))
                    stf = strip_pool.tile([128, M], F32, tag="stripf",
                                          name=f"stripf_{s}_{hh}_{mc}")
                    for r in range(2):
                        nc.vector.reciprocal_approx_fast(
                            out=stf[:, 512 * r:512 * (r + 1)], in_=zps[r])
                    st = strip_pool.tile([128, M], BF16, tag="strip",
                                         name=f"strip_{s}_{hh}_{mc}")
                    nc.scalar.activation(out=st, in_=stf, func=COPY)
                    strips[u] = st

                Rsb = {}

                def sel_pair(u, n0, n1, s=s):
                    """R broadcast: sel matmul -> PSUM, ACT copy -> SBUF bf16."""
                    hh, mc = units[u]
                    for nt in (n0, n1):
                        r, a = nt // 4, nt % 4
                        if probe >= 2:
                            continue
                        r_ps = zr_pool.tile([128, 512], F32, tag="zr",
                                            name=f"rps_{s}_{hh}_{mc}_{nt}")
                        nc.tensor.matmul(
                            r_ps, sel_sb[32 * a:32 * a + 4, :],
                            strips[u][32 * a:32 * a + 4, 512 * r:512 * (r + 1)],
                            start=True, stop=True,
                            tile_position=(32 * a, 0))
                        if rsb:
                            t = rsb_pool.tile([128, 512], BF16, tag="rsb",
                                              name=f"rsb_{s}_{hh}_{mc}_{nt}")
                            nc.scalar.activation(out=t, in_=r_ps, func=COPY)
                            Rsb[u, nt] = t
                        else:
                            Rsb[u, nt] = r_ps

                def mult_pv_pair(u, n0, n1, s=s):
                    hh, mc = units[u]
                    h = 2 * s + hh
                    mcs = slice(512 * mc, 512 * (mc + 1))
                    for nt in (n0, n1):
                        nc.vector.tensor_mul(
                            out=E[s, hh, nt][:, mcs],
                            in0=E[s, hh, nt][:, mcs],
                            in1=probe_ones if probe else Rsb[u, nt])
                    for nt in (n0, n1):
                        nc.tensor.matmul(
                            pv_tiles[s][64 * hh:64 * (hh + 1), mcs],
                            v_sb[nt][:, 64 * h:64 * (h + 1)],
                            E[s, hh, nt][:, mcs],
                            start=(nt == 0), stop=(nt == 7))

                def sel_one(u, nt, s=s):
                    hh, mc = units[u]
                    r, a = nt // 4, nt % 4
                    if probe >= 2:
                        return
                    r_ps = zr_pool.tile([128, 512], F32, tag="zr",
                                        name=f"rps_{s}_{hh}_{mc}_{nt}")
                    nc.tensor.matmul(
                        r_ps, sel_sb[32 * a:32 * a + 4, :],
                        strips[u][32 * a:32 * a + 4, 512 * r:512 * (r + 1)],
                        start=True, stop=True,
                        tile_position=(32 * a, 0))
                    Rsb[u, nt] = r_ps

                def mult_pv_one(u, nt, s=s):
                    hh, mc = units[u]
                    h = 2 * s + hh
                    mcs = slice(512 * mc, 512 * (mc + 1))
                    nc.vector.tensor_mul(
                        out=E[s, hh, nt][:, mcs],
                        in0=E[s, hh, nt][:, mcs],
                        in1=probe_ones if probe else Rsb[u, nt])
                    nc.tensor.matmul(
                        pv_tiles[s][64 * hh:64 * (hh + 1), mcs],
                        v_sb[nt][:, 64 * h:64 * (h + 1)],
                        E[s, hh, nt][:, mcs],
                        start=(nt == 0), stop=(nt == 7))

                if pairgrp:
                    pairs = [(0, 4), (1, 5), (2, 6), (3, 7)]
                    for u in range(4):
                        chunks.append(lambda u=u: zrc(u))
                        for pi, (n0, n1) in enumerate(pairs):
                            chunks.append(
                                lambda u=u, n0=n0, n1=n1: sel_pair(u, n0, n1))
                            if pi >= 1:
                                p0, p1 = pairs[pi - 1]
                                chunks.append(
                                    lambda u=u, p0=p0, p1=p1: mult_pv_pair(u, p0, p1))
                        chunks.append(
                            lambda u=u: mult_pv_pair(u, pairs[3][0], pairs[3][1]))
                else:
                    ntseq = (0, 4, 1, 5, 2, 6, 3, 7)
                    for u in range(4):
                        chunks.append(lambda u=u: zrc(u))
                        for i, nt in enumerate(ntseq):
                            chunks.append(lambda u=u, nt=nt: sel_one(u, nt))
                            if i >= 1:
                                chunks.append(
                                    lambda u=u, p=ntseq[i - 1]: mult_pv_one(u, p))
                        chunks.append(lambda u=u: mult_pv_one(u, ntseq[7]))

                def copyout(s=s):
                    nc.scalar.activation(out=outT_sb[s], in_=pv_tiles[s],
                                         func=COPY)
                chunks.append(copyout)
                return chunks

            # software-pipeline: alpha(s) interleaved with beta(s-1);
            # s=0 interleaves the remaining front work instead
            npair = 4
            for s in range(npair + 1):
                a = alpha_chunks(s) if s < npair and parts != "front" else []
                mk_beta = beta_chunks_mc if mcmerge else beta_chunks
                if s == 0:
                    b = front_rest
                else:
                    b = mk_beta(s - 1) if parts in ("fab", "all") else []
                na, nb = len(a), len(b)
                if not a:
                    for f in b:
                        f()
                else:
                    ratio = nb / na if na else 0
                    bi = 0.0
                    for i, f in enumerate(a):
                        f()
                        target = (i + 1) * ratio
                        while bi < target and int(bi) < nb:
                            b[int(bi)]()
                            bi += 1
                    for j in range(int(bi), nb):
                        b[j]()

        # ================= PROJ (yT = w_out.T-contract @ outT) =================
        if parts != "all":
            # keep y written so the output tensor exists
            with ExitStack() as pj:
                y_pool = pj.enter_context(tc.tile_pool(name="y0_sb", bufs=1))
                y_sb = y_pool.tile([128, M], BF16, tag="y0")
                nc.vector.memset(y_sb, 0.0)
                for dc in range(4):
                    nc.sync.dma_start(out=y[128 * dc:128 * (dc + 1), :], in_=y_sb)
        elif True:
          with ExitStack() as pj:
            pj_pool = pj.enter_context(tc.tile_pool(name="pj_ps", bufs=2, space="PSUM"))
            y_pool = pj.enter_context(tc.tile_pool(name="y_sb", bufs=2))
            for dc in range(4):
                y_sb = y_pool.tile([128, M], BF16, tag="y")
                for mc in range(2):
                    mcs = slice(512 * mc, 512 * (mc + 1))
                    p = pj_pool.tile([128, 512], F32, tag="pj")
                    for kt in range(4):
                        nc.tensor.matmul(p,
                                         wo_sb[kt][:, 128 * dc:128 * (dc + 1)],
                                         outT_sb[kt][:, 512 * mc:512 * (mc + 1)],
                                         start=(kt == 0), stop=(kt == 3))
                    nc.scalar.activation(out=y_sb[:, mcs], in_=p, func=COPY)
                    # store each half as soon as it is ready (spreads queues)
                    nc.sync.dma_start(out=y[128 * dc:128 * (dc + 1), mcs],
                                      in_=y_sb[:, mcs])

    nc.compile()
    return nc


def _consts():
    import ml_dtypes
    bf16 = ml_dtypes.bfloat16
    ident = np.eye(128, dtype=np.float32).astype(bf16)
    # i4big[:, 128a:128(a+1)] maps E-tile partition p -> strip row 32a + p//32
    i4big = np.zeros((128, 512), np.float32)
    for a in range(4):
        for p in range(128):
            i4big[p, 128 * a + 32 * a + p // 32] = 1.0
    i4big_bf = i4big.astype(bf16)
    sel = np.zeros((128, 128), np.float32)
    for p in range(128):
        if p % 32 < 4:
            for c in range(128):
                if c // 32 == p % 32:
                    sel[p, c] = 1.0
    return ident, i4big_bf, sel.astype(bf16)


def _in_maps(x, w_qkv, w_out):
    import ml_dtypes
    bf16 = ml_dtypes.bfloat16
    ident, i4big, sel = _consts()
    x = np.asarray(x, dtype=np.float32)
    wq = np.asarray(w_qkv, np.float32).astype(bf16)
    wo = np.asarray(w_out, np.float32).astype(bf16)
    maps = []
    for c in range(8):
        maps.append({
            "x": np.ascontiguousarray(x[c].reshape(M, DIM)).astype(bf16),
            "w_qkv": wq, "w_out": wo,
            "ident": ident, "i4big": i4big,
            "i4big32": i4big.astype(np.float32), "sel": sel,
        })
    return maps


def kernel(x, w_qkv, w_out, b_out):
    from concourse import bass_utils
    if "nc" not in _CACHE:
        _CACHE["nc"] = _build()
    nc = _CACHE["nc"]
    in_maps = _in_maps(x, w_qkv, w_out)
    res = bass_utils.run_bass_kernel_spmd(nc, in_maps, core_ids=list(range(8)))
    b = np.asarray(b_out, np.float32).reshape(1, DIM)
    out = np.stack([
        (np.asarray(res.results[c]["y"], np.float32).T + b).reshape(H, W, DIM)
        for c in range(8)
    ])
    return out
